# revision 1
# baseline (speedup 1.0000x reference)
"""Trainium2 Bass kernel for nn_AttentionBlock (B=4, C=1024, T=1024, H=16).

Sharding: data-parallel over batch (4) x sequence-parallel over T (2 halves)
= 8 cores, zero collectives. k/v are computed for the full sequence on every
core (attention needs all keys); q/softmax/attention-output/projection only
for the core's T-half. Per-core inputs are T-permuted on the host so the
SPMD program always works on columns [0, 512).

Numerics: matmuls run in bf16 (weights cast on host, activations rounded
on-chip) with fp32 PSUM accumulation; GroupNorm statistics and softmax
normalization stay fp32. bf16 enables PE tile_position packing for the
K=64 / M=64 attention matmuls (fp32r cannot pack).

Schedule notes: DMA triggers cost ~700ns each on the issuing engine's
queue, so they are spread across sync/vector/scalar (x tiles, interleaved
with the GroupNorm work that consumes them) and gpsimd (weights+consts).
The exp activations pace the attention phase (~1.3us per [128,1024] tile
on the scalar engine), so k-projection matmul groups are braided between
score chunks of the preceding attention pair to keep the PE busy while
exp catches up.
"""

import numpy as np

C, T, TH = 1024, 1024, 512
H, CH = 16, 64
NG, GS = 32, 32  # groups, channels per group
EPS = 1e-5
B = 4
NCORES = 8
NP = 8  # head pairs
SCALE2 = 1.0 / 8.0  # (ch^-0.25)^2 applied to q.k scores

_NC = None  # compiled Bass module cache
_LAST_RESULTS = None


def _build_bass():
    import concourse.bacc as bacc
    import concourse.tile as tile
    from concourse import mybir
    from contextlib import ExitStack

    F32 = mybir.dt.float32
    BF16 = mybir.dt.bfloat16
    AF = mybir.ActivationFunctionType
    nc = bacc.Bacc(None, target_bir_lowering=False)

    x_d = nc.dram_tensor("x", [C, T], F32, kind="ExternalInput")
    wq_d = nc.dram_tensor("wq", [C, C], BF16, kind="ExternalInput")
    wk_d = nc.dram_tensor("wk", [C, C], BF16, kind="ExternalInput")
    wv_d = nc.dram_tensor("wv", [C, C], BF16, kind="ExternalInput")
    pw_d = nc.dram_tensor("pw", [C, C], BF16, kind="ExternalInput")
    bq_d = nc.dram_tensor("bq", [128, 8], F32, kind="ExternalInput")
    bk_d = nc.dram_tensor("bk", [128, 8], F32, kind="ExternalInput")
    bv_d = nc.dram_tensor("bv", [128, 8], F32, kind="ExternalInput")
    pb_d = nc.dram_tensor("pb", [128, 8], F32, kind="ExternalInput")
    nw_d = nc.dram_tensor("nw", [128, 8], F32, kind="ExternalInput")
    nb_d = nc.dram_tensor("nb", [128, 8], F32, kind="ExternalInput")
    comb_d = nc.dram_tensor("comb", [128, 4], F32, kind="ExternalInput")
    gbc_d = nc.dram_tensor("gbc", [4, 128], F32, kind="ExternalInput")
    sel2_d = nc.dram_tensor("sel2", [33, 128], BF16, kind="ExternalInput")
    out_d = nc.dram_tensor("out", [C, TH], F32, kind="ExternalOutput")

    with tile.TileContext(nc) as tc, ExitStack() as glob:
        gpool = glob.enter_context(tc.tile_pool(name="gpool", bufs=1))

        # ---- persistent small tiles (triggered on gpsimd: idle queue) ---
        comb_s = gpool.tile([128, 4], F32, name="comb_s")
        gbc_s = gpool.tile([4, 128], F32, name="gbc_s")
        sel2_s = gpool.tile([33, 128], BF16, name="sel2_s")
        eps4 = gpool.tile([4, 1], F32, name="eps4")
        bq_all = gpool.tile([128, 8], F32, name="bq_all")
        bk_all = gpool.tile([128, 8], F32, name="bk_all")
        bv_all = gpool.tile([128, 8], F32, name="bv_all")
        pb_all = gpool.tile([128, 8], F32, name="pb_all")
        nw_all = gpool.tile([128, 8], F32, name="nw_all")
        nb_all = gpool.tile([128, 8], F32, name="nb_all")
        rc2_t = [gpool.tile([33, TH], BF16, name=f"rc2_{p}") for p in range(2)]
        # norm scale/bias first: the GroupNorm tail needs them earliest
        nc.gpsimd.dma_start(out=nw_all, in_=nw_d[:, :])
        nc.gpsimd.dma_start(out=nb_all, in_=nb_d[:, :])
        nc.gpsimd.dma_start(out=comb_s, in_=comb_d[:, :])
        nc.gpsimd.dma_start(out=gbc_s, in_=gbc_d[:, :])
        nc.gpsimd.dma_start(out=bq_all, in_=bq_d[:, :])
        nc.gpsimd.dma_start(out=bk_all, in_=bk_d[:, :])
        nc.gpsimd.dma_start(out=bv_all, in_=bv_d[:, :])
        nc.gpsimd.dma_start(out=pb_all, in_=pb_d[:, :])
        nc.gpsimd.dma_start(out=sel2_s, in_=sel2_d[:, :])
        nc.vector.memset(eps4, EPS)
        nc.vector.memset(rc2_t[0], 0.0)
        nc.vector.memset(rc2_t[1], 0.0)

        # ---- persistent activation tiles --------------------------------
        xnstack = glob.enter_context(ExitStack())
        xnpool = xnstack.enter_context(tc.tile_pool(name="xnpool", bufs=1, side="right"))
        xn = [xnpool.tile([128, T], BF16, name=f"xn{i}", tag=f"xn{i}") for i in range(8)]
        q_s = [gpool.tile([128, TH], BF16, name=f"q{j}", tag=f"q{j}") for j in range(NP)]
        kk = [gpool.tile([128, T], BF16, name=f"kk{j}", tag=f"kk{j}") for j in range(NP)]
        vaA = [gpool.tile([128, 8, 65], BF16, name=f"vaA{t}", tag=f"vaA{t}")
               for t in range(8)]
        vaB = [gpool.tile([128, 8, 128], BF16, name=f"vaB{t}", tag=f"vaB{t}")
               for t in range(8)]
        a_all = [gpool.tile([128, TH], BF16, name=f"a{c}", tag=f"a{c}") for c in range(8)]
        for t in range(8):
            # fused softmax-denominator columns: ones in v feed the row-sum
            nc.vector.memset(vaA[t][:, :, 64:65], 1.0)
            nc.vector.memset(vaB[t][:, :, 0:1], 1.0)
            nc.vector.memset(vaB[t][:, :, 1:64], 0.0)

        wstack = glob.enter_context(ExitStack())
        wpool = wstack.enter_context(tc.tile_pool(name="wpool", bufs=1, side="right"))
        wq_b = wpool.tile([128, 8, C], BF16, name="wq_b")
        wk_b = wpool.tile([128, 8, C], BF16, name="wk_b")
        wv_b = wpool.tile([128, 8, C], BF16, name="wv_b")
        # weight loads on the gpsimd queue (after consts), q weights first
        for c in range(8):
            nc.gpsimd.dma_start(out=wq_b[:, c, :], in_=wq_d[128 * c:128 * c + 128, :])
        for c in range(8):
            nc.gpsimd.dma_start(out=wk_b[:, c, :], in_=wk_d[128 * c:128 * c + 128, :])
        for c in range(8):
            nc.gpsimd.dma_start(out=wv_b[:, c, :], in_=wv_d[128 * c:128 * c + 128, :])

        # ---- phase 1: GroupNorm, per-tile (pipelines with x DMA) --------
        # each 128-channel tile holds 4 complete norm groups, so its stats
        # are self-contained; x-tile DMA triggers are spread across three
        # engine queues so the last tile is triggered by ~6us.
        ph23 = glob.enter_context(ExitStack())
        mm_ps = ph23.enter_context(tc.tile_pool(name="mm_ps", bufs=2, space="PSUM"))
        qkv_stack = glob.enter_context(ExitStack())
        mm_b = qkv_stack.enter_context(tc.tile_pool(name="mm_b", bufs=2, space="PSUM"))
        mm_c = qkv_stack.enter_context(tc.tile_pool(name="mm_c", bufs=2, space="PSUM"))
        with ExitStack() as ph1:
            xpool = ph1.enter_context(tc.tile_pool(name="xpool", bufs=8))
            spool = ph1.enter_context(tc.tile_pool(name="spool", bufs=2))
            gn_ps = ph1.enter_context(tc.tile_pool(name="gn_ps", bufs=1, space="PSUM"))
            bc_ps = ph1.enter_context(tc.tile_pool(name="bc_ps", bufs=1, space="PSUM"))
            for i in range(8):
                r0 = 128 * i
                x_t = xpool.tile([128, T], F32, tag="x_t", name=f"x_t{i}")
                eng = nc.sync if i % 2 == 0 else nc.scalar
                eng.dma_start(out=x_t, in_=x_d[r0:r0 + 128, :])
                st = spool.tile([128, 2, 6], F32, tag="st", name=f"st{i}")
                nc.vector.bn_stats(out=st[:, 0, :], in_=x_t[:, 0:512])
                nc.vector.bn_stats(out=st[:, 1, :], in_=x_t[:, 512:1024])
                mv = spool.tile([128, 2], F32, tag="mv", name=f"mv{i}")
                nc.vector.bn_aggr(out=mv, in_=st)
                # mq = [mean, E[x^2]] per channel
                mq = spool.tile([128, 2], F32, tag="mq", name=f"mq{i}")
                nc.vector.tensor_mul(mq[:, 1:2], mv[:, 0:1], mv[:, 0:1])
                nc.vector.tensor_add(mq[:, 1:2], mq[:, 1:2], mv[:, 1:2])
                nc.vector.tensor_copy(mq[:, 0:1], mv[:, 0:1])
                gst = gn_ps.tile([4, 2], F32, tag="gst", name=f"gst{i}")
                nc.tensor.matmul(gst, comb_s, mq, start=True, stop=True)
                gsb = spool.tile([4, 2], F32, tag="gsb", name=f"gsb{i}")
                nc.vector.tensor_copy(gsb, gst)
                var4 = spool.tile([4, 1], F32, tag="var4", name=f"var4{i}")
                nc.vector.tensor_mul(var4, gsb[:, 0:1], gsb[:, 0:1])
                nc.vector.tensor_sub(var4, gsb[:, 1:2], var4)
                rs = spool.tile([4, 2], F32, tag="rs", name=f"rs{i}")
                nc.scalar.activation(out=rs[:, 1:2], in_=var4, func=AF.Sqrt,
                                     bias=eps4, scale=1.0)
                nc.vector.reciprocal(rs[:, 1:2], rs[:, 1:2])
                nc.vector.tensor_copy(rs[:, 0:1], gsb[:, 0:1])
                bc = bc_ps.tile([128, 2], F32, tag="bc", name=f"bc{i}")
                nc.tensor.matmul(bc, gbc_s, rs, start=True, stop=True)
                sca = spool.tile([128, 1], F32, tag="sca", name=f"sca{i}")
                sha = spool.tile([128, 1], F32, tag="sha", name=f"sha{i}")
                nc.vector.tensor_mul(sca, bc[:, 1:2], nw_all[:, i:i + 1])
                nc.vector.tensor_mul(sha, bc[:, 0:1], sca)
                nc.vector.tensor_sub(sha, nb_all[:, i:i + 1], sha)
                nc.vector.tensor_scalar(out=xn[i], in0=x_t,
                                        scalar1=sca, scalar2=sha,
                                        op0=mybir.AluOpType.mult,
                                        op1=mybir.AluOpType.add)

        # ---- phase 2: QKV projections -----------------------------------
        def q_group(j):
            qp = (mm_ps, mm_b, mm_c)[j % 3].tile(
                [128, TH], F32, tag="qkv", name=f"qp{j}")
            for c in range(8):
                nc.tensor.matmul(qp, wq_b[:, c, 128 * j:128 * j + 128],
                                 xn[c][:, 0:TH], start=(c == 0), stop=(c == 7))
            nc.vector.tensor_scalar_add(q_s[j], qp, bq_all[:, j:j + 1])

        def k_group_mms(j, sn, kp, c0, c1):
            for c in range(c0, c1):
                nc.tensor.matmul(kp, wk_b[:, c, 128 * j:128 * j + 128],
                                 xn[c][:, TH * sn:TH * sn + TH],
                                 start=(c == 0), stop=(c == 7))
            if c1 == 8:
                nc.vector.tensor_scalar_add(kk[j][:, TH * sn:TH * sn + TH],
                                            kp, bk_all[:, j:j + 1])

        def k_group(j, sn):
            kp = (mm_b, mm_c)[sn].tile(
                [128, TH], F32, tag="qkv", name=f"kp{j}_{sn}")
            k_group_mms(j, sn, kp, 0, 8)

        for j in range(NP):
            q_group(j)
        k_group(0, 0)
        k_group(0, 1)
        k_group(1, 0)
        k_group(1, 1)
        qkv_stack.close()

        # ---- attention pools (gn/bc pools closed; 2+4+1+1 = 8 banks) ----
        qk_ps = ph23.enter_context(tc.tile_pool(name="qk_ps", bufs=2, space="PSUM"))
        av_ps = ph23.enter_context(tc.tile_pool(name="av_ps", bufs=1, space="PSUM"))
        epool = ph23.enter_context(tc.tile_pool(name="epool", bufs=10))
        rpool = ph23.enter_context(tc.tile_pool(name="rpool", bufs=2))

        def score_chunk(j, sc):
            qk = qk_ps.tile([128, 2, TH], F32, tag="qk", name=f"qk{j}_{sc}")
            nc.tensor.matmul(qk[:, 0, :], kk[j][0:64, 128 * sc:128 * sc + 128],
                             q_s[j][0:64, :], start=True, stop=True)
            nc.tensor.matmul(qk[:, 1, :], kk[j][64:128, 128 * sc:128 * sc + 128],
                             q_s[j][64:128, :], start=True, stop=True)
            et = epool.tile([128, 2, TH], BF16, tag="et", name=f"et{j}_{sc}")
            nc.scalar.activation(out=et, in_=qk, func=AF.Exp, scale=SCALE2)
            return et

        def av_tiles(j):
            return (av_ps.tile([128, TH], F32, tag="avA", name=f"avA{j}"),
                    av_ps.tile([128, TH], F32, tag="avB", name=f"avB{j}"))

        def av_step(j, avt, ets, sc):
            avA, avB = avt
            st_, sp_ = (sc == 0), (sc == 7)
            nc.tensor.matmul(avA[0:65, :], vaA[sc][:, j, :],
                             ets[sc][:, 0, :], start=st_, stop=sp_)
            nc.tensor.matmul(avB, vaB[sc][:, j, :],
                             ets[sc][:, 1, :], start=st_, stop=sp_)

        def attn_finish(j, avt):
            avA, avB = avt
            dd = rpool.tile([33, TH], F32, tag="dd", name=f"dd{j}")
            nc.vector.tensor_copy(dd[0:1, :], avA[64:65, :])
            nc.vector.tensor_copy(dd[32:33, :], avB[0:1, :])
            rcp = rpool.tile([33, TH], F32, tag="rcp", name=f"rcp{j}")
            nc.vector.reciprocal_approx_fast(out=rcp, in_=dd)
            rc2 = rc2_t[j % 2]
            with nc.allow_low_precision(reason="bf16 feed for PE broadcast"):
                nc.vector.tensor_copy(rc2[0:1, :], rcp[0:1, :])
                nc.vector.tensor_copy(rc2[32:33, :], rcp[32:33, :])
            db = mm_ps.tile([128, TH], F32, tag="qkv", name=f"db{j}")
            nc.tensor.matmul(db, sel2_s, rc2, start=True, stop=True)
            at_ = a_all[j]
            with nc.allow_low_precision(reason="bf16 attention output"):
                nc.vector.tensor_copy(at_[0:64, :], avA[0:64, :])
                nc.vector.tensor_copy(at_[64:128, :], avB[64:128, :])
                nc.vector.tensor_mul(at_, at_, db)
            nc.vector.tensor_scalar_add(at_, at_, bv_all[:, j:j + 1])

        # v production braided with pair-0 score chunks so exp starts early
        ets0 = []
        for tt in range(8):
            for h2 in range(2):
                vp = mm_ps.tile([128, TH], F32, tag="qkv", name=f"vp{tt}_{h2}")
                for c in range(8):
                    nc.tensor.matmul(vp, xn[c][:, 128 * tt:128 * tt + 128],
                                     wv_b[:, c, TH * h2:TH * h2 + TH],
                                     start=(c == 0), stop=(c == 7))
                vpv = vp.rearrange("p (j c) -> p j c", c=64)
                j0 = 4 * h2
                with nc.allow_low_precision(reason="bf16 v for attention"):
                    nc.vector.tensor_copy(vaA[tt][:, j0:j0 + 4, 0:64], vpv[:, 0::2, :])
                    nc.vector.tensor_copy(vaB[tt][:, j0:j0 + 4, 64:128], vpv[:, 1::2, :])
            ets0.append(score_chunk(0, tt))
        prev_j, prev_ets, prev_avt = 0, ets0, av_tiles(0)

        # xn freed after the braided k groups below complete; prefetch proj
        # weights on gpsimd meanwhile
        pwpool = glob.enter_context(tc.tile_pool(name="pwpool", bufs=1, side="left"))
        pw_b = pwpool.tile([128, 8, C], BF16, name="pw_b")
        for c in range(8):
            nc.gpsimd.dma_start(out=pw_b[:, c, :], in_=pw_d[128 * c:128 * c + 128, :])
        xrpool = glob.enter_context(tc.tile_pool(name="xrpool", bufs=8))
        xr_t = []
        for ot in range(8):
            xr = xrpool.tile([128, TH], F32, tag="xr", name=f"xr{ot}")
            nc.gpsimd.dma_start(out=xr, in_=x_d[128 * ot:128 * ot + 128, 0:TH])
            xr_t.append(xr)

        # attention pairs 1..7; k-projection groups for pairs 2..7 are
        # braided between score chunks (2 matmuls per chunk) so the PE has
        # work while the scalar engine's exp activations catch up.
        for j in range(1, NP):
            braid = j + 1 if j + 1 < NP else None
            kps = None
            if braid is not None:
                kps = [mm_ps.tile([128, TH], F32, tag="qkv", name=f"kp{braid}_{sn}")
                       for sn in range(2)]
            ets = []
            for sc in range(8):
                ets.append(score_chunk(j, sc))
                if braid is not None:
                    sn, cb = divmod(sc, 4)
                    k_group_mms(braid, sn, kps[sn], 2 * cb, 2 * cb + 2)
                av_step(prev_j, prev_avt, prev_ets, sc)
            attn_finish(prev_j, prev_avt)
            prev_j, prev_ets, prev_avt = j, ets, av_tiles(j)

        # drain the last pair; braid one projection group over chunks 0..6
        # between its attention-output steps
        hp_pre = [mm_ps.tile([128, TH], F32, tag="qkv", name="hp0")]
        for sc in range(8):
            av_step(prev_j, prev_avt, prev_ets, sc)
            if sc < 7:
                nc.tensor.matmul(hp_pre[0], pw_b[:, sc, 0:128],
                                 a_all[sc], start=(sc == 0), stop=False)
        attn_finish(prev_j, prev_avt)

        wstack.close()
        xnstack.close()

        # ---- phase 3: project, residual ---------------------------------
        with ExitStack() as ph3:
            opool = ph3.enter_context(tc.tile_pool(name="opool", bufs=3))
            for ot in range(8):
                r0 = 128 * ot
                if ot < 1:
                    hp = hp_pre[ot]
                    nc.tensor.matmul(hp, pw_b[:, 7, r0:r0 + 128],
                                     a_all[7], start=False, stop=True)
                else:
                    hp = mm_ps.tile([128, TH], F32, tag="qkv", name=f"hp{ot}")
                    for c in range(8):
                        nc.tensor.matmul(hp, pw_b[:, c, r0:r0 + 128],
                                         a_all[c], start=(c == 0), stop=(c == 7))
                o_t = opool.tile([128, TH], F32, tag="o_t", name=f"o_t{ot}")
                nc.vector.scalar_tensor_tensor(o_t, hp, pb_all[:, ot:ot + 1], xr_t[ot],
                                               op0=mybir.AluOpType.add,
                                               op1=mybir.AluOpType.add)
                nc.gpsimd.dma_start(out=out_d[r0:r0 + 128, 0:256], in_=o_t[:, 0:256])
                nc.gpsimd.dma_start(out=out_d[r0:r0 + 128, 256:512], in_=o_t[:, 256:512])

    nc.finalize()
    return nc


def kernel(x, norm_weight, norm_bias, qkv_w, qkv_b, proj_w, proj_b):
    from concourse.bass_utils import run_bass_kernel_spmd
    import ml_dtypes

    global _NC
    if _NC is None:
        _NC = _build_bass()

    BF = ml_dtypes.bfloat16
    x = np.ascontiguousarray(np.asarray(x, dtype=np.float32))
    nw = np.asarray(norm_weight, np.float32)
    nb = np.asarray(norm_bias, np.float32)
    qw = np.asarray(qkv_w, np.float32).reshape(H, 3, CH, C)
    qb = np.asarray(qkv_b, np.float32).reshape(H, 3, CH)
    pw = np.asarray(proj_w, np.float32)
    pb = np.asarray(proj_b, np.float32)

    wq = np.ascontiguousarray(qw[:, 0].reshape(C, C).T.astype(BF))
    wk = np.ascontiguousarray(qw[:, 1].reshape(C, C).T.astype(BF))
    wv = np.ascontiguousarray(qw[:, 2].reshape(C, C).T.astype(BF))
    pwT = np.ascontiguousarray(pw.T.astype(BF))

    def b128(v):
        return np.ascontiguousarray(np.asarray(v, np.float32).reshape(8, 128).T)

    comb = np.zeros((128, 4), np.float32)
    for p in range(128):
        comb[p, p // 32] = 1.0 / 32.0
    gbc = np.zeros((4, 128), np.float32)
    for p in range(128):
        gbc[p // 32, p] = 1.0
    sel2 = np.zeros((33, 128), BF)
    sel2[0, 0:64] = 1
    sel2[32, 64:128] = 1

    shared = dict(wq=wq, wk=wk, wv=wv, pw=pwT,
                  bq=b128(qb[:, 0].reshape(C)), bk=b128(qb[:, 1].reshape(C)),
                  bv=b128(qb[:, 2].reshape(C)), pb=b128(pb),
                  nw=b128(nw), nb=b128(nb), comb=comb, gbc=gbc,
                  sel2=sel2)
    in_maps = []
    for core in range(NCORES):
        b, half = divmod(core, 2)
        xb = x[b] if half == 0 else np.ascontiguousarray(
            np.concatenate([x[b][:, TH:], x[b][:, :TH]], axis=1))
        in_maps.append(dict(x=xb, **shared))

    import os
    kw = {}
    if os.environ.get("BASS_KERNEL_TRACE"):
        cores = os.environ.get("BASS_KERNEL_TRACE_CORES", "0")
        kw = dict(trace=True,
                  trace_cores=[int(c) for c in cores.split(",")],
                  stitch_traces=len(cores.split(",")) > 1)
    res = run_bass_kernel_spmd(_NC, in_maps, core_ids=list(range(NCORES)), **kw)
    global _LAST_RESULTS
    _LAST_RESULTS = res
    out = np.empty((B, C, T), np.float32)
    for core in range(NCORES):
        b, half = divmod(core, 2)
        out[b][:, half * TH:(half + 1) * TH] = res.results[core]["out"]
    return out

